# revision 52
# baseline (speedup 1.0000x reference)
"""Trainium2 Bass kernel for nn_MultiHeadedAttention_4604204941604.

Multi-headed attention with a distance-MLP reweighting term:
  out = ((softmax(mask(QK^T/8)) * distMLP(d)^2) masked) @ V @ Wo

Host-side structural simplifications (carried over from v1):

1. MLP collapse: the distance-MLP biases are all zero and
   src_distances >= 0, so the MLP collapses to dist = C * d with a
   scalar C computed on the host (validity asserted).

2. Mask compaction: rows/keys with mask==0 are compacted out on the
   host; each core's own query rows come FIRST in key order so the
   score diagonal sits at fixed positions for every core.

v2 on-device restructure (vs. v1):

* Scores are computed TRANSPOSED (keys on psum partitions, queries on
  the free axis): ssT[k, q] = k_h . q_h.  This removes all 80 PE
  transposes of p and the qt row-split: one N=NQP moving pass per
  128-key chunk.  p^T is then natively laid out for the PV matmul
  (oo = v_chunk^T @ p_unT accumulated over key chunks).
* The softmax denominator is a ones-stationary matmul over e^T chunks
  (partition-dim reduction on the PE), returned to the host.
* No on-device normalization: the kernel returns per-head unnormalized
  z_h = Wo_h^T @ num_h ([512, NQP]) plus den_h; the host computes
  out = sum_h z_h^T / den_h.  Division/pad correction are host-side.
* DMA issue order = K-proj inputs, V, Q, distances, Wo, so projections
  start as soon as their operands land (completion tracks issue order).
* A short PE warm-up burst spans the initial DMA window so HAM
  un-throttles the PE clock (1.2 -> 2.4 GHz) before the projections.

Sharding: core c handles batch b = c//4, query rows 256*(c%4)..+256.
"""

import os
import sys
import types

sys.path.insert(0, "/opt/trn_rl_repo")

import numpy as np
import ml_dtypes

import concourse.bass as bass
import concourse.bacc as bacc
import concourse.mybir as mybir
from concourse import tile
from concourse.masks import make_identity

BF16 = mybir.dt.bfloat16
F32 = mybir.dt.float32
NPBF16 = ml_dtypes.bfloat16

B, N, D, H = 2, 1024, 512, 8
DK = D // H  # 64
NCORES = 8
RPC = N * B // NCORES  # 256 query rows per core
NEG = -1e8

_cache = {}


def _install_ntff_hook():
    try:
        from antenv.axon_hooks import get_axon_ntff_profile_hook  # noqa: F401
        return
    except ImportError:
        pass
    import antenv
    mod = types.ModuleType("antenv.axon_hooks")
    _hook = [None]
    mod.set_axon_ntff_profile_hook = lambda h: _hook.__setitem__(0, h)
    mod.get_axon_ntff_profile_hook = lambda: _hook[0]
    sys.modules["antenv.axon_hooks"] = mod
    antenv.axon_hooks = mod
    try:
        from trn_agent_boot.trn_boot import _ntff_profile_via_ctypes
        mod.set_axon_ntff_profile_hook(
            _ntff_profile_via_ctypes("/opt/axon/libaxon_pjrt.so"))
    except Exception:
        pass


def _chunk_layout(NQP, NKP):
    """Key chunks (kc0, kcn) and their (bank, col-offset) inside the
    [128, 2, 512] packed score/e/p layout: 3 chunks in bank 0, 2 in
    bank 1, each NQP wide."""
    assert 128 < NQP <= 170, NQP   # 3*NQP must fit a 512-f32 psum bank
    assert 512 < NKP <= 640, NKP   # exactly 5 key chunks
    KCH = [(c0, min(128, NKP - c0)) for c0 in range(0, NKP, 128)]
    assert len(KCH) == 5
    pos = [(0, 0), (0, NQP), (0, 2 * NQP), (1, 0), (1, NQP)]
    return KCH, pos


def _build_program(NQP, NKP):
    KCH, POS = _chunk_layout(NQP, NKP)
    nc = bacc.Bacc("TRN2", target_bir_lowering=False, debug=False)

    d_qT = nc.dram_tensor("qT", (D, NQP), BF16, kind="ExternalInput")
    d_kT = nc.dram_tensor("kT", (D, NKP), BF16, kind="ExternalInput")
    d_vT = nc.dram_tensor("vT", (D, NKP), BF16, kind="ExternalInput")
    d_dist = nc.dram_tensor("dist", (128, 2, 512), BF16, kind="ExternalInput")
    d_c128 = nc.dram_tensor("c128", (128, 1), F32, kind="ExternalInput")
    d_wq = nc.dram_tensor("wq", (D, D), BF16, kind="ExternalInput")
    d_wk = nc.dram_tensor("wk", (D, D), BF16, kind="ExternalInput")
    d_wv = nc.dram_tensor("wv", (D, D), BF16, kind="ExternalInput")
    d_wo8 = nc.dram_tensor("wo8", (2 * DK, H // 2, D), BF16, kind="ExternalInput")
    d_ebias = nc.dram_tensor("ebias", (128, 2), F32, kind="ExternalInput")
    d_out = nc.dram_tensor("out", (128, 2, 512), BF16, kind="ExternalOutput")
    DBG = os.environ.get("BASS_DEBUG_DUMP", "0") == "1"
    if DBG:
        d_dbg_kt = nc.dram_tensor("dbg_kt", (128, 4, NKP), BF16,
                                  kind="ExternalOutput")
        d_dbg_qt = nc.dram_tensor("dbg_qt", (128, 4, NQP), BF16,
                                  kind="ExternalOutput")
        d_dbg_sc = nc.dram_tensor("dbg_sc", (128, 2, 512), F32,
                                  kind="ExternalOutput")
        d_dbg_e = nc.dram_tensor("dbg_e", (128, 2, 512), BF16,
                                 kind="ExternalOutput")

    with tile.TileContext(nc) as tc:
        with (
            tc.tile_pool(name="const", bufs=1) as cp,
            tc.tile_pool(name="esb", bufs=6) as ep,
            tc.tile_pool(name="pusb", bufs=2) as pup,
            tc.tile_pool(name="pusb2", bufs=2) as pup2,
            tc.tile_pool(name="rsbp", bufs=5) as rsbp,
            tc.tile_pool(name="oosb", bufs=2) as oop_sb,
            tc.tile_pool(name="osb", bufs=1) as osb_p,
            tc.tile_pool(name="big", bufs=2, space=bass.MemorySpace.PSUM) as bigp,
            tc.tile_pool(name="dn", bufs=1, space=bass.MemorySpace.PSUM) as dnp,
            tc.tile_pool(name="oo", bufs=1, space=bass.MemorySpace.PSUM) as oop,
            tc.tile_pool(name="ot", bufs=1, space=bass.MemorySpace.PSUM) as otp,
        ):
            ident = cp.tile([128, 128], BF16, tag="ident")
            warm = cp.tile([128, 512], BF16, tag="warm")
            nc.vector.memset(warm[:], 0.0)
            make_identity(nc, ident[:])
            ones128 = cp.tile([128, 128], BF16, tag="ones128")
            nc.vector.memset(ones128[:], 1.0)
            ebias = cp.tile([128, 2], F32, tag="ebias")
            nc.sync.dma_start(ebias[:], d_ebias[:])
            # (1 - I) masks that zero the self-attention diagonal of e^T
            # (own queries are keys 0..NQP in key order)
            nq1 = NQP - 128
            m01 = cp.tile([128, 2 * NQP], BF16, tag="m01")
            nc.vector.memset(m01[:], 1.0)
            nc.vector.tensor_sub(m01[:, :128], m01[:, :128], ident[:])
            nc.vector.tensor_sub(m01[:nq1, NQP + 128:2 * NQP],
                                 m01[:nq1, NQP + 128:2 * NQP],
                                 ident[:nq1, :nq1])
            c128 = cp.tile([128, 1], F32, tag="c128")
            nc.sync.dma_start(c128[:], d_c128[:])

            kTin = cp.tile([128, 4, NKP], BF16, tag="kTin")
            vTin = cp.tile([128, 4, NKP], BF16, tag="vTin")
            qTin = cp.tile([128, 4, NQP], BF16, tag="qTin")
            wq = cp.tile([128, 4, D], BF16, tag="wq")
            wk = cp.tile([128, 4, D], BF16, tag="wk")
            wv = cp.tile([128, 4, D], BF16, tag="wv")
            # DMA issue order tracks completion order: K-proj inputs
            # first (split fine across queues), then V, then Q.
            hk = NKP // 2
            for j in range(4):
                for s in range(2):
                    nc.gpsimd.dma_start(
                        kTin[:, j, s * hk:(s + 1) * hk],
                        d_kT.rearrange("(j p) n -> p j n", p=128)[:, j, s * hk:(s + 1) * hk])
                    nc.sync.dma_start(
                        wk[:, j, s * 256:(s + 1) * 256],
                        d_wk.rearrange("(j p) n -> p j n", p=128)[:, j, s * 256:(s + 1) * 256])
            for j in range(4):
                nc.gpsimd.dma_start(qTin[:, j, :], d_qT.rearrange("(j p) n -> p j n", p=128)[:, j, :])
                nc.sync.dma_start(wq[:, j, :], d_wq.rearrange("(j p) n -> p j n", p=128)[:, j, :])
            distpk = cp.tile([128, 2, 512], BF16, tag="distpk")
            nc.gpsimd.dma_start(distpk[:], d_dist[:])
            wo8 = cp.tile([2 * DK, H // 2, D], BF16, tag="wo8")
            nc.sync.dma_start(wo8[:, :2], d_wo8[:, :2])
            for j in range(4):
                nc.gpsimd.dma_start(vTin[:, j, :], d_vT.rearrange("(j p) n -> p j n", p=128)[:, j, :])
                nc.sync.dma_start(wv[:, j, :], d_wv.rearrange("(j p) n -> p j n", p=128)[:, j, :])
            nc.sync.dma_start(wo8[:, 2:], d_wo8[:, 2:])

            # q projections with the other head-half zeroed, so the
            # scores matmul contracts over the full 128 partitions
            qTz = [cp.tile([128, 4, NQP], BF16, tag=f"qTz{x}", name=f"qTz{x}") for x in range(2)]
            nc.vector.memset(qTz[0][:], 0.0)
            nc.vector.memset(qTz[1][:], 0.0)
            kT = cp.tile([128, 4, NKP], BF16, tag="kTp")
            v = cp.tile([128, 5, D], BF16, tag="vp")
            d2T = cp.tile([128, 2, 512], BF16, tag="d2T")
            oosbz = [cp.tile([128, NQP], BF16, tag=f"oosbz{x}", name=f"oosbz{x}") for x in range(2)]
            nc.vector.memset(oosbz[0][:], 0.0)
            nc.vector.memset(oosbz[1][:], 0.0)

            # PE warm-up burst over the input-DMA window: HAM
            # un-throttles after ~3.4us of sustained activity.
            wps = bigp.tile([128, 2, 512], F32, tag="big")
            for _ in range(18):
                nc.tensor.matmul(wps[:, 0, :], warm[:, :128], warm[:],
                                 start=True, stop=True)
            wsink = cp.tile([128, 1], F32, tag="wsink")
            nc.vector.tensor_copy(wsink[:], wps[:, 0, :1])

            SPC = [(0, 0, 512), (1, 0, NKP - 512)]

            def proj_k(i):
                ps = bigp.tile([128, 2, 512], F32, tag="big")
                for bk, c0, cn in SPC:
                    for j in range(4):
                        nc.tensor.matmul(
                            ps[:, bk, c0:c0 + cn],
                            wk[:, j, 128 * i:128 * i + 128],
                            kTin[:, j, 512 * bk + c0:512 * bk + c0 + cn],
                            start=(j == 0), stop=(j == 3))
                nc.vector.tensor_copy(kT[:, i, :512], ps[:, 0, :])
                nc.vector.tensor_copy(kT[:, i, 512:NKP], ps[:, 1, :NKP - 512])

            def proj_v(c):
                kc0, kcn = KCH[c]
                ps = bigp.tile([128, 2, 512], F32, tag="big")
                for j in range(4):
                    nc.tensor.matmul(ps[:kcn, 0, :], vTin[:, j, kc0:kc0 + kcn],
                                     wv[:, j, :], start=(j == 0), stop=(j == 3))
                nc.vector.tensor_copy(v[:kcn, c, :], ps[:kcn, 0, :])

            def proj_q(i):
                ps = bigp.tile([128, 2, 512], F32, tag="big")
                for j in range(4):
                    nc.tensor.matmul(ps[:, 0, :NQP], wq[:, j, 128 * i:128 * i + 128],
                                     qTin[:, j, :], start=(j == 0), stop=(j == 3))
                nc.scalar.copy(qTz[0][:64, i, :], ps[:64, 0, :NQP])
                nc.scalar.copy(qTz[1][64:, i, :], ps[64:, 0, :NQP])

            # outT accumulates over all 8 heads with start=False
            # matmuls; pre-zero its two banks (full-bank start=True
            # writes of zeros) so the first accumulate lands on 0.
            # This sidesteps the bank-wide has_written clear that a
            # start=True matmul performs.
            outT = otp.tile([128, 2, 512], F32, tag="outT")
            for b in range(2):
                nc.tensor.matmul(outT[:, b, :], warm[:, :128], warm[:],
                                 start=True, stop=False,
                                 skip_group_check=True)

            def stage_a(h):
                ch = h // 2
                # scores^T: ssT[k, q] = k_h . q_h per 128-key chunk.
                # Full-128 contraction: the other head-half of the moving
                # q is zero, so the whole PE array stays active.
                sc = bigp.tile([128, 2, 512], F32, tag="big")
                for c, (kc0, kcn) in enumerate(KCH):
                    bk, off = POS[c]
                    nc.tensor.matmul(sc[:kcn, bk, off:off + NQP],
                                     kT[:, ch, kc0:kc0 + kcn],
                                     qTz[h % 2][:, ch, :],
                                     start=True, stop=True)
                return sc

            def stage_b(h, sc):
                e = ep.tile([128, 2, 512], BF16, tag="e")
                nc.scalar.activation(e[:, 0, :3 * NQP], sc[:, 0, :3 * NQP],
                                     mybir.ActivationFunctionType.Exp,
                                     bias=0.0, scale=0.125)
                nc.scalar.activation(e[:, 1, :NQP], sc[:, 1, :NQP],
                                     mybir.ActivationFunctionType.Exp,
                                     bias=ebias[:, 0:1], scale=0.125)
                nc.scalar.activation(e[:64, 1, NQP:2 * NQP],
                                     sc[:64, 1, NQP:2 * NQP],
                                     mybir.ActivationFunctionType.Exp,
                                     bias=ebias[:64, 1:2], scale=0.125)

                # diag-masked e for the chunks containing own queries
                # (one op: chunks 0 and 1 are contiguous in e's bank 0)
                em = ep.tile([128, 2 * NQP], BF16, tag="em")
                nc.vector.tensor_mul(em[:], e[:, 0, 0:2 * NQP], m01[:])

                return e, em

            def stage_den(h, eem):
                e, em = eem
                # den[q] = sum_k e[k, q], broadcast over 128 psum
                # partitions by a [kcn, 128] all-ones stationary.
                dn = dnp.tile([128, NQP], F32, tag="dn")
                for c, (kc0, kcn) in enumerate(KCH):
                    bk, off = POS[c]
                    mov = (em[:kcn, c * NQP:(c + 1) * NQP] if c < 2
                           else e[:kcn, bk, off:off + NQP])
                    nc.tensor.matmul(dn[:, :], ones128[:kcn, :], mov,
                                     start=(c == 0), stop=(c == 4))
                rsf = pup.tile([128, NQP], F32, tag="rsf")
                nc.vector.reciprocal_approx_fast(rsf[:], dn[:, :])
                rsb = rsbp.tile([128, NQP], BF16, tag="rsb")
                if h % 2 == 0:
                    nc.scalar.copy(rsb[:], rsf[:])
                else:
                    nc.vector.tensor_copy(rsb[:], rsf[:])
                return rsb

            def stage_pu(h, eem):
                e, em = eem
                # unnormalized p^T = e * (C d)^2
                pu = pup2.tile([128, 2, 512], BF16, tag="pu")
                nc.vector.tensor_mul(pu[:, 0, :3 * NQP], e[:, 0, :3 * NQP],
                                     d2T[:, 0, :3 * NQP])
                nc.vector.tensor_mul(pu[:, 1, :2 * NQP], e[:, 1, :2 * NQP],
                                     d2T[:, 1, :2 * NQP])
                return pu

            def stage_pv(h, pu, rsb):
                # num^T for the whole head PAIR block: stationary is the
                # 128-wide v column block of heads (2*ch, 2*ch+1); only
                # this head's 64 output rows are used downstream.
                ch = h // 2
                pr = 64 * (h % 2)
                oo = oop.tile([128, NQP], F32, tag="oo")
                for c, (kc0, kcn) in enumerate(KCH):
                    bk, off = POS[c]
                    nc.tensor.matmul(oo[:, :], v[:kcn, c, 128 * ch:128 * ch + 128],
                                     pu[:kcn, bk, off:off + NQP],
                                     start=(c == 0), stop=(c == 4))

                # normalize while copying out of psum: num / den.  The
                # destination is a head-parity tile whose other half is
                # permanently zero, so the out-projection below can
                # contract over the full 128 partitions.
                oosb = oosbz[h % 2]
                nc.vector.tensor_mul(oosb[pr:pr + 64, :], oo[pr:pr + 64, :],
                                     rsb[:DK, :])

            def stage_z(h):
                # out^T[o, q] += Wo_pair^T @ (num_h / den_h), 4 o-chunks.
                # Lagged one stage behind stage_c so the PE never waits
                # on the normalize chain.
                ch = h // 2
                for oc in range(4):
                    bk, off = oc // 2, (oc % 2) * NQP
                    nc.tensor.matmul(outT[:, bk, off:off + NQP],
                                     wo8[:, ch, 128 * oc:128 * oc + 128],
                                     oosbz[h % 2][:], start=False,
                                     stop=(h == H - 1),
                                     skip_group_check=True)

            # Phase 1: K and Q projections while inputs stream; then
            # the score/exp/den front halves of heads 0-3 (independent
            # of V) run while the V inputs land; then the V projection;
            # then the PV back halves interleaved with heads 4-7.
            for i in range(4):
                proj_k(i)
            for i in range(4):
                proj_q(i)
            eems = {}
            rsbs = {}
            for h in range(4):
                eems[h] = stage_b(h, stage_a(h))
                rsbs[h] = stage_den(h, eems[h])
                if h == 3:
                    # (C*d)^2; emitted here so the ACT queue reaches it
                    # only after the dist DMA has landed
                    nc.scalar.activation(d2T[:, 0, :], distpk[:, 0, :],
                                         mybir.ActivationFunctionType.Square,
                                         bias=0.0, scale=c128[:])
                    nc.scalar.activation(d2T[:, 1, :], distpk[:, 1, :],
                                         mybir.ActivationFunctionType.Square,
                                         bias=0.0, scale=c128[:])
            for c in range(5):
                proj_v(c)
            for h in range(4):
                stage_pv(h, stage_pu(h, eems[h]), rsbs[h])
                stage_z(h)
                eems[h + 4] = stage_b(h + 4, stage_a(h + 4))
            for h in range(4, H):
                rsbs[h] = stage_den(h, eems[h])
                stage_pv(h, stage_pu(h, eems[h]), rsbs[h])
                stage_z(h)

            outsb = osb_p.tile([128, 2, 2 * NQP], BF16, tag="osb")
            nc.vector.tensor_copy(outsb[:, 0, :NQP], outT[:, 0, :NQP])
            nc.scalar.copy(outsb[:, 0, NQP:2 * NQP], outT[:, 0, NQP:2 * NQP])
            nc.vector.tensor_copy(outsb[:, 1, :NQP], outT[:, 1, :NQP])
            nc.scalar.copy(outsb[:, 1, NQP:2 * NQP], outT[:, 1, NQP:2 * NQP])
            nc.sync.dma_start(d_out[:, :, :2 * NQP], outsb[:])

    nc.compile()
    return nc


def _get_program(nqp, nkp):
    key = ("prog", nqp, nkp)
    if key not in _cache:
        _cache[key] = _build_program(nqp, nkp)
    return _cache[key]


def kernel(**inputs):
    from concourse import bass_utils

    query = np.asarray(inputs["query"], np.float32)
    key = np.asarray(inputs["key"], np.float32)
    value = np.asarray(inputs["value"], np.float32)
    dist = np.asarray(inputs["src_distances"], np.float32)
    mask = np.asarray(inputs["mask"])
    dW1, db1 = np.asarray(inputs["dW1"], np.float64), np.asarray(inputs["db1"])
    dW2, db2 = np.asarray(inputs["dW2"], np.float64), np.asarray(inputs["db2"])
    dW3, db3 = np.asarray(inputs["dW3"], np.float64), np.asarray(inputs["db3"])
    dW4, db4 = np.asarray(inputs["dW4"], np.float64), np.asarray(inputs["db4"])

    assert all(np.all(b == 0) for b in (db1, db2, db3, db4)), \
        "distance-MLP collapse requires zero biases"
    assert dist.min() >= 0.0, "distance-MLP collapse requires d >= 0"
    u = np.maximum(dW1[0], 0.0)
    u = np.maximum(u @ dW2, 0.0)
    u = np.maximum(u @ dW3, 0.0)
    C = float(u @ dW4[:, 0])

    wq_b = np.asarray(inputs["Wq"], np.float32).astype(NPBF16)
    wk_b = np.asarray(inputs["Wk"], np.float32).astype(NPBF16)
    wv_b = np.asarray(inputs["Wv"], np.float32).astype(NPBF16)
    wo = np.asarray(inputs["Wo"], np.float32)
    wo8 = np.ascontiguousarray(
        wo.reshape(H // 2, 2 * DK, D).transpose(1, 0, 2)).astype(NPBF16)
    c128 = np.full((128, 1), C, np.float32)

    mf = mask != 0
    nq_max = max(int(mf[c // 4, RPC * (c % 4):RPC * (c % 4) + RPC].sum())
                 for c in range(NCORES))
    nv_max = max(int(mf[b].sum()) for b in range(B))
    NQP = max(144, 128 + ((nq_max - 128 + 15) // 16) * 16)
    NKP = max(544, 512 + ((nv_max - 512 + 31) // 32) * 32)
    KCH, POS = _chunk_layout(NQP, NKP)

    in_maps = []
    qidx_all = []
    npads = []
    for c in range(NCORES):
        b, r0 = c // 4, RPC * (c % 4)
        qidx = np.nonzero(mf[b, r0:r0 + RPC])[0]  # local valid query rows
        kid_own = r0 + qidx                       # global, matches q order
        other = np.nonzero(mf[b])[0]
        other = other[(other < r0) | (other >= r0 + RPC)]
        korder = np.concatenate([kid_own, other])
        nq, nv = len(qidx), len(korder)
        assert nq <= NQP and nv <= NKP, (nq, nv)
        qidx_all.append(qidx)
        npads.append(NKP - nv)

        qTh = np.zeros((D, NQP), NPBF16)
        qTh[:, :nq] = query[b, r0 + qidx].T.astype(NPBF16)
        kTh = np.zeros((D, NKP), NPBF16)
        kTh[:, :nv] = key[b, korder].T.astype(NPBF16)
        vTh = np.zeros((D, NKP), NPBF16)
        vTh[:, :nv] = value[b, korder].T.astype(NPBF16)
        # distances, transposed [key, query], packed into [128, 2, 512]
        dT = np.zeros((640, NQP), np.float32)
        dT[:nv, :nq] = dist[b, r0 + qidx][:, korder].T
        dpk = np.zeros((128, 2, 512), NPBF16)
        for ci, (kc0, kcn) in enumerate(KCH):
            bk, off = POS[ci]
            dpk[:kcn, bk, off:off + NQP] = dT[kc0:kc0 + kcn].astype(NPBF16)
        # zero the self-attention diagonal (key j == query j for j < NQP)
        for j in range(128):
            dpk[j, 0, j] = 0
        for p in range(NQP - 128):
            dpk[p, 0, NQP + 128 + p] = 0
        # per-key exp bias: -30 knocks zero-padded keys out of the
        # denominator (exp(0 - 30) ~ 0); chunks 0-2 are always valid
        assert nv >= 384, nv
        ebias = np.zeros((128, 2), np.float32)
        keys3 = 384 + np.arange(128)
        keys4 = 512 + np.arange(128)
        ebias[:, 0] = np.where(keys3 < nv, 0.0, -30.0)
        ebias[:, 1] = np.where(keys4 < nv, 0.0, -30.0)
        in_maps.append({
            "qT": qTh, "kT": kTh, "vT": vTh, "dist": dpk, "c128": c128,
            "wq": wq_b, "wk": wk_b, "wv": wv_b, "wo8": wo8, "ebias": ebias,
        })

    trace = os.environ.get("BASS_KERNEL_TRACE", "0") == "1"
    if trace:
        _install_ntff_hook()

    prog = _get_program(NQP, NKP)
    res = bass_utils.run_bass_kernel_spmd(
        prog, in_maps, core_ids=list(range(NCORES)), trace=trace)

    out = np.zeros((B, N, D), np.float32)
    for c in range(NCORES):
        b, r0 = c // 4, RPC * (c % 4)
        qidx = qidx_all[c]
        nq = len(qidx)
        ot = np.asarray(res.results[c]["out"], np.float32)  # [128, 2, 512]
        oT = np.concatenate(
            [ot[:, 0, 0:NQP], ot[:, 0, NQP:2 * NQP],
             ot[:, 1, 0:NQP], ot[:, 1, NQP:2 * NQP]], axis=0)  # [512, NQP]
        out[b, r0 + qidx] = oT[:, :nq].T
    kernel.last_exec_time_ns = res.exec_time_ns
    kernel._last_res = res
    kernel._last_meta = (NQP, NKP, qidx_all, npads, in_maps)
    return out


kernel.last_exec_time_ns = None


# revision 53
# speedup vs baseline: 1.0022x; 1.0022x over previous
"""Trainium2 Bass kernel for nn_MultiHeadedAttention_4604204941604.

Multi-headed attention with a distance-MLP reweighting term:
  out = ((softmax(mask(QK^T/8)) * distMLP(d)^2) masked) @ V @ Wo

Host-side structural simplifications (carried over from v1):

1. MLP collapse: the distance-MLP biases are all zero and
   src_distances >= 0, so the MLP collapses to dist = C * d with a
   scalar C computed on the host (validity asserted).

2. Mask compaction: rows/keys with mask==0 are compacted out on the
   host; each core's own query rows come FIRST in key order so the
   score diagonal sits at fixed positions for every core.

v2 on-device restructure (vs. v1):

* Scores are computed TRANSPOSED (keys on psum partitions, queries on
  the free axis): ssT[k, q] = k_h . q_h.  This removes all 80 PE
  transposes of p and the qt row-split: one N=NQP moving pass per
  128-key chunk.  p^T is then natively laid out for the PV matmul
  (oo = v_chunk^T @ p_unT accumulated over key chunks).
* The softmax denominator is a ones-stationary matmul over e^T chunks
  (partition-dim reduction on the PE), returned to the host.
* No on-device normalization: the kernel returns per-head unnormalized
  z_h = Wo_h^T @ num_h ([512, NQP]) plus den_h; the host computes
  out = sum_h z_h^T / den_h.  Division/pad correction are host-side.
* DMA issue order = K-proj inputs, V, Q, distances, Wo, so projections
  start as soon as their operands land (completion tracks issue order).
* A short PE warm-up burst spans the initial DMA window so HAM
  un-throttles the PE clock (1.2 -> 2.4 GHz) before the projections.

Sharding: core c handles batch b = c//4, query rows 256*(c%4)..+256.
"""

import os
import sys
import types

sys.path.insert(0, "/opt/trn_rl_repo")

import numpy as np
import ml_dtypes

import concourse.bass as bass
import concourse.bacc as bacc
import concourse.mybir as mybir
from concourse import tile
from concourse.masks import make_identity

BF16 = mybir.dt.bfloat16
F32 = mybir.dt.float32
NPBF16 = ml_dtypes.bfloat16

B, N, D, H = 2, 1024, 512, 8
DK = D // H  # 64
NCORES = 8
RPC = N * B // NCORES  # 256 query rows per core
NEG = -1e8

_cache = {}


def _install_ntff_hook():
    try:
        from antenv.axon_hooks import get_axon_ntff_profile_hook  # noqa: F401
        return
    except ImportError:
        pass
    import antenv
    mod = types.ModuleType("antenv.axon_hooks")
    _hook = [None]
    mod.set_axon_ntff_profile_hook = lambda h: _hook.__setitem__(0, h)
    mod.get_axon_ntff_profile_hook = lambda: _hook[0]
    sys.modules["antenv.axon_hooks"] = mod
    antenv.axon_hooks = mod
    try:
        from trn_agent_boot.trn_boot import _ntff_profile_via_ctypes
        mod.set_axon_ntff_profile_hook(
            _ntff_profile_via_ctypes("/opt/axon/libaxon_pjrt.so"))
    except Exception:
        pass


def _chunk_layout(NQP, NKP):
    """Key chunks (kc0, kcn) and their (bank, col-offset) inside the
    [128, 2, 512] packed score/e/p layout: 3 chunks in bank 0, 2 in
    bank 1, each NQP wide."""
    assert 128 < NQP <= 170, NQP   # 3*NQP must fit a 512-f32 psum bank
    assert 512 < NKP <= 640, NKP   # exactly 5 key chunks
    KCH = [(c0, min(128, NKP - c0)) for c0 in range(0, NKP, 128)]
    assert len(KCH) == 5
    pos = [(0, 0), (0, NQP), (0, 2 * NQP), (1, 0), (1, NQP)]
    return KCH, pos


def _build_program(NQP, NKP):
    KCH, POS = _chunk_layout(NQP, NKP)
    nc = bacc.Bacc("TRN2", target_bir_lowering=False, debug=False)

    d_qT = nc.dram_tensor("qT", (D, NQP), BF16, kind="ExternalInput")
    d_kT = nc.dram_tensor("kT", (D, NKP), BF16, kind="ExternalInput")
    d_vT = nc.dram_tensor("vT", (D, NKP), BF16, kind="ExternalInput")
    d_dist = nc.dram_tensor("dist", (128, 2, 512), BF16, kind="ExternalInput")
    d_c128 = nc.dram_tensor("c128", (128, 1), F32, kind="ExternalInput")
    d_wq = nc.dram_tensor("wq", (D, D), BF16, kind="ExternalInput")
    d_wk = nc.dram_tensor("wk", (D, D), BF16, kind="ExternalInput")
    d_wv = nc.dram_tensor("wv", (D, D), BF16, kind="ExternalInput")
    d_wo8 = nc.dram_tensor("wo8", (2 * DK, H // 2, D), BF16, kind="ExternalInput")
    d_ebias = nc.dram_tensor("ebias", (128, 2), F32, kind="ExternalInput")
    d_out = nc.dram_tensor("out", (128, 2, 512), BF16, kind="ExternalOutput")
    DBG = os.environ.get("BASS_DEBUG_DUMP", "0") == "1"
    if DBG:
        d_dbg_kt = nc.dram_tensor("dbg_kt", (128, 4, NKP), BF16,
                                  kind="ExternalOutput")
        d_dbg_qt = nc.dram_tensor("dbg_qt", (128, 4, NQP), BF16,
                                  kind="ExternalOutput")
        d_dbg_sc = nc.dram_tensor("dbg_sc", (128, 2, 512), F32,
                                  kind="ExternalOutput")
        d_dbg_e = nc.dram_tensor("dbg_e", (128, 2, 512), BF16,
                                 kind="ExternalOutput")

    with tile.TileContext(nc) as tc:
        with (
            tc.tile_pool(name="const", bufs=1) as cp,
            tc.tile_pool(name="esb", bufs=6) as ep,
            tc.tile_pool(name="pusb", bufs=2) as pup,
            tc.tile_pool(name="pusb2", bufs=2) as pup2,
            tc.tile_pool(name="rsbp", bufs=5) as rsbp,
            tc.tile_pool(name="oosb", bufs=2) as oop_sb,
            tc.tile_pool(name="osb", bufs=1) as osb_p,
            tc.tile_pool(name="big", bufs=2, space=bass.MemorySpace.PSUM) as bigp,
            tc.tile_pool(name="dn", bufs=1, space=bass.MemorySpace.PSUM) as dnp,
            tc.tile_pool(name="oo", bufs=1, space=bass.MemorySpace.PSUM) as oop,
            tc.tile_pool(name="ot", bufs=1, space=bass.MemorySpace.PSUM) as otp,
        ):
            ident = cp.tile([128, 128], BF16, tag="ident")
            warm = cp.tile([128, 512], BF16, tag="warm")
            nc.vector.memset(warm[:], 0.0)
            make_identity(nc, ident[:])
            ones128 = cp.tile([128, 128], BF16, tag="ones128")
            nc.vector.memset(ones128[:], 1.0)
            ebias = cp.tile([128, 2], F32, tag="ebias")
            nc.sync.dma_start(ebias[:], d_ebias[:])
            # (1 - I) masks that zero the self-attention diagonal of e^T
            # (own queries are keys 0..NQP in key order)
            nq1 = NQP - 128
            m01 = cp.tile([128, 2 * NQP], BF16, tag="m01")
            nc.vector.memset(m01[:], 1.0)
            nc.vector.tensor_sub(m01[:, :128], m01[:, :128], ident[:])
            nc.vector.tensor_sub(m01[:nq1, NQP + 128:2 * NQP],
                                 m01[:nq1, NQP + 128:2 * NQP],
                                 ident[:nq1, :nq1])
            c128 = cp.tile([128, 1], F32, tag="c128")
            nc.sync.dma_start(c128[:], d_c128[:])

            kTin = cp.tile([128, 4, NKP], BF16, tag="kTin")
            vTin = cp.tile([128, 4, NKP], BF16, tag="vTin")
            qTin = cp.tile([128, 4, NQP], BF16, tag="qTin")
            wq = cp.tile([128, 4, D], BF16, tag="wq")
            wk = cp.tile([128, 4, D], BF16, tag="wk")
            wv = cp.tile([128, 4, D], BF16, tag="wv")
            # DMA issue order tracks completion order: K-proj inputs
            # first (split fine across queues), then V, then Q.
            hk = NKP // 2
            for j in range(4):
                for s in range(2):
                    nc.gpsimd.dma_start(
                        kTin[:, j, s * hk:(s + 1) * hk],
                        d_kT.rearrange("(j p) n -> p j n", p=128)[:, j, s * hk:(s + 1) * hk])
                    nc.sync.dma_start(
                        wk[:, j, s * 256:(s + 1) * 256],
                        d_wk.rearrange("(j p) n -> p j n", p=128)[:, j, s * 256:(s + 1) * 256])
            for j in range(4):
                nc.gpsimd.dma_start(qTin[:, j, :], d_qT.rearrange("(j p) n -> p j n", p=128)[:, j, :])
                nc.sync.dma_start(wq[:, j, :], d_wq.rearrange("(j p) n -> p j n", p=128)[:, j, :])
            distpk = cp.tile([128, 2, 512], BF16, tag="distpk")
            nc.gpsimd.dma_start(distpk[:], d_dist[:])
            wo8 = cp.tile([2 * DK, H // 2, D], BF16, tag="wo8")
            nc.sync.dma_start(wo8[:, :2], d_wo8[:, :2])
            for j in range(4):
                nc.gpsimd.dma_start(vTin[:, j, :], d_vT.rearrange("(j p) n -> p j n", p=128)[:, j, :])
                nc.sync.dma_start(wv[:, j, :], d_wv.rearrange("(j p) n -> p j n", p=128)[:, j, :])
            nc.sync.dma_start(wo8[:, 2:], d_wo8[:, 2:])

            # q projections with the other head-half zeroed, so the
            # scores matmul contracts over the full 128 partitions
            qTz = [cp.tile([128, 4, NQP], BF16, tag=f"qTz{x}", name=f"qTz{x}") for x in range(2)]
            nc.vector.memset(qTz[0][:], 0.0)
            nc.vector.memset(qTz[1][:], 0.0)
            kT = cp.tile([128, 4, NKP], BF16, tag="kTp")
            v = cp.tile([128, 5, D], BF16, tag="vp")
            d2T = cp.tile([128, 2, 512], BF16, tag="d2T")
            oosbz = [cp.tile([128, NQP], BF16, tag=f"oosbz{x}", name=f"oosbz{x}") for x in range(2)]
            nc.vector.memset(oosbz[0][:], 0.0)
            nc.vector.memset(oosbz[1][:], 0.0)

            # PE warm-up burst over the input-DMA window: HAM
            # un-throttles after ~3.4us of sustained activity.
            wps = bigp.tile([128, 2, 512], F32, tag="big")
            for _ in range(18):
                nc.tensor.matmul(wps[:, 0, :], warm[:, :128], warm[:],
                                 start=True, stop=True)
            wsink = cp.tile([128, 1], F32, tag="wsink")
            nc.vector.tensor_copy(wsink[:], wps[:, 0, :1])

            SPC = [(0, 0, 512), (1, 0, NKP - 512)]

            def proj_k(i):
                ps = bigp.tile([128, 2, 512], F32, tag="big")
                for bk, c0, cn in SPC:
                    for j in range(4):
                        nc.tensor.matmul(
                            ps[:, bk, c0:c0 + cn],
                            wk[:, j, 128 * i:128 * i + 128],
                            kTin[:, j, 512 * bk + c0:512 * bk + c0 + cn],
                            start=(j == 0), stop=(j == 3))
                nc.vector.tensor_copy(kT[:, i, :512], ps[:, 0, :])
                nc.vector.tensor_copy(kT[:, i, 512:NKP], ps[:, 1, :NKP - 512])

            def proj_v(c):
                kc0, kcn = KCH[c]
                ps = bigp.tile([128, 2, 512], F32, tag="big")
                for j in range(4):
                    nc.tensor.matmul(ps[:kcn, 0, :], vTin[:, j, kc0:kc0 + kcn],
                                     wv[:, j, :], start=(j == 0), stop=(j == 3))
                nc.vector.tensor_copy(v[:kcn, c, :], ps[:kcn, 0, :])

            def proj_q(i):
                ps = bigp.tile([128, 2, 512], F32, tag="big")
                for j in range(4):
                    nc.tensor.matmul(ps[:, 0, :NQP], wq[:, j, 128 * i:128 * i + 128],
                                     qTin[:, j, :], start=(j == 0), stop=(j == 3))
                nc.scalar.copy(qTz[0][:64, i, :], ps[:64, 0, :NQP])
                nc.scalar.copy(qTz[1][64:, i, :], ps[64:, 0, :NQP])

            # outT accumulates over all 8 heads with start=False
            # matmuls; pre-zero its two banks (full-bank start=True
            # writes of zeros) so the first accumulate lands on 0.
            # This sidesteps the bank-wide has_written clear that a
            # start=True matmul performs.
            outT = otp.tile([128, 2, 512], F32, tag="outT")
            for b in range(2):
                nc.tensor.matmul(outT[:, b, :], warm[:, :128], warm[:],
                                 start=True, stop=False,
                                 skip_group_check=True)

            def stage_a(h):
                ch = h // 2
                # scores^T: ssT[k, q] = k_h . q_h per 128-key chunk.
                # Full-128 contraction: the other head-half of the moving
                # q is zero, so the whole PE array stays active.
                sc = bigp.tile([128, 2, 512], F32, tag="big")
                for c, (kc0, kcn) in enumerate(KCH):
                    bk, off = POS[c]
                    nc.tensor.matmul(sc[:kcn, bk, off:off + NQP],
                                     kT[:, ch, kc0:kc0 + kcn],
                                     qTz[h % 2][:, ch, :],
                                     start=True, stop=True)
                return sc

            def stage_b(h, sc):
                e = ep.tile([128, 2, 512], BF16, tag="e")
                nc.scalar.activation(e[:, 0, :3 * NQP], sc[:, 0, :3 * NQP],
                                     mybir.ActivationFunctionType.Exp,
                                     bias=0.0, scale=0.125)
                nc.scalar.activation(e[:, 1, :NQP], sc[:, 1, :NQP],
                                     mybir.ActivationFunctionType.Exp,
                                     bias=ebias[:, 0:1], scale=0.125)
                nc.scalar.activation(e[:64, 1, NQP:2 * NQP],
                                     sc[:64, 1, NQP:2 * NQP],
                                     mybir.ActivationFunctionType.Exp,
                                     bias=ebias[:64, 1:2], scale=0.125)

                # diag-masked e for the chunks containing own queries
                # (one op: chunks 0 and 1 are contiguous in e's bank 0)
                em = ep.tile([128, 2 * NQP], BF16, tag="em")
                nc.vector.tensor_mul(em[:], e[:, 0, 0:2 * NQP], m01[:])

                return e, em

            def stage_den(h, eem):
                e, em = eem
                # den[q] = sum_k e[k, q], broadcast over 128 psum
                # partitions by a [kcn, 128] all-ones stationary.
                dn = dnp.tile([128, NQP], F32, tag="dn")
                for c, (kc0, kcn) in enumerate(KCH):
                    bk, off = POS[c]
                    mov = (em[:kcn, c * NQP:(c + 1) * NQP] if c < 2
                           else e[:kcn, bk, off:off + NQP])
                    nc.tensor.matmul(dn[:, :], ones128[:kcn, :], mov,
                                     start=(c == 0), stop=(c == 4))
                rsf = pup.tile([128, NQP], F32, tag="rsf")
                nc.vector.reciprocal_approx_fast(rsf[:], dn[:, :])
                rsb = rsbp.tile([128, NQP], BF16, tag="rsb")
                if h % 2 == 0:
                    nc.scalar.copy(rsb[:], rsf[:])
                else:
                    nc.vector.tensor_copy(rsb[:], rsf[:])
                return rsb

            def stage_pu(h, eem):
                e, em = eem
                # unnormalized p^T = e * (C d)^2
                pu = pup2.tile([128, 2, 512], BF16, tag="pu")
                nc.vector.tensor_mul(pu[:, 0, :3 * NQP], e[:, 0, :3 * NQP],
                                     d2T[:, 0, :3 * NQP])
                nc.vector.tensor_mul(pu[:, 1, :2 * NQP], e[:, 1, :2 * NQP],
                                     d2T[:, 1, :2 * NQP])
                return pu

            def stage_pv(h, pu, rsb):
                # num^T for the whole head PAIR block: stationary is the
                # 128-wide v column block of heads (2*ch, 2*ch+1); only
                # this head's 64 output rows are used downstream.
                ch = h // 2
                pr = 64 * (h % 2)
                oo = oop.tile([128, NQP], F32, tag="oo")
                for c, (kc0, kcn) in enumerate(KCH):
                    bk, off = POS[c]
                    nc.tensor.matmul(oo[:, :], v[:kcn, c, 128 * ch:128 * ch + 128],
                                     pu[:kcn, bk, off:off + NQP],
                                     start=(c == 0), stop=(c == 4))

                # normalize while copying out of psum: num / den.  The
                # destination is a head-parity tile whose other half is
                # permanently zero, so the out-projection below can
                # contract over the full 128 partitions.
                oosb = oosbz[h % 2]
                nc.vector.tensor_mul(oosb[pr:pr + 64, :], oo[pr:pr + 64, :],
                                     rsb[:DK, :])

            def stage_z(h):
                # out^T[o, q] += Wo_pair^T @ (num_h / den_h), 4 o-chunks.
                # Lagged one stage behind stage_c so the PE never waits
                # on the normalize chain.
                ch = h // 2
                for oc in range(4):
                    bk, off = oc // 2, (oc % 2) * NQP
                    nc.tensor.matmul(outT[:, bk, off:off + NQP],
                                     wo8[:, ch, 128 * oc:128 * oc + 128],
                                     oosbz[h % 2][:], start=False,
                                     stop=(h == H - 1),
                                     skip_group_check=True)

            # Phase 1: K and Q projections while inputs stream; then
            # the score/exp/den front halves of heads 0-3 (independent
            # of V) run while the V inputs land; then the V projection;
            # then the PV back halves interleaved with heads 4-7.
            for i in range(4):
                proj_k(i)
            for i in range(4):
                proj_q(i)
            eems = {}
            rsbs = {}
            for h in range(4):
                eems[h] = stage_b(h, stage_a(h))
                rsbs[h] = stage_den(h, eems[h])
                if h == 3:
                    # (C*d)^2; emitted here so the ACT queue reaches it
                    # only after the dist DMA has landed
                    nc.scalar.activation(d2T[:, 0, :], distpk[:, 0, :],
                                         mybir.ActivationFunctionType.Square,
                                         bias=0.0, scale=c128[:])
                    nc.scalar.activation(d2T[:, 1, :], distpk[:, 1, :],
                                         mybir.ActivationFunctionType.Square,
                                         bias=0.0, scale=c128[:])
            for c in range(5):
                proj_v(c)
            for h in range(4):
                stage_pv(h, stage_pu(h, eems[h]), rsbs[h])
                stage_z(h)
                eems[h + 4] = stage_b(h + 4, stage_a(h + 4))
            for h in range(4, H):
                rsbs[h] = stage_den(h, eems[h])
                stage_pv(h, stage_pu(h, eems[h]), rsbs[h])
                stage_z(h)

            outsb = osb_p.tile([128, 2, 2 * NQP], BF16, tag="osb")
            nc.vector.tensor_copy(outsb[:, 0, :NQP], outT[:, 0, :NQP])
            nc.scalar.copy(outsb[:, 0, NQP:2 * NQP], outT[:, 0, NQP:2 * NQP])
            nc.vector.tensor_copy(outsb[:, 1, :NQP], outT[:, 1, :NQP])
            nc.scalar.copy(outsb[:, 1, NQP:2 * NQP], outT[:, 1, NQP:2 * NQP])
            nc.sync.dma_start(d_out[:, :, :2 * NQP], outsb[:])

    nc.compile()
    return nc


def _get_program(nqp, nkp):
    key = ("prog", nqp, nkp)
    if key not in _cache:
        _cache[key] = _build_program(nqp, nkp)
    return _cache[key]


def kernel(**inputs):
    from concourse import bass_utils

    query = np.asarray(inputs["query"], np.float32)
    key = np.asarray(inputs["key"], np.float32)
    value = np.asarray(inputs["value"], np.float32)
    dist = np.asarray(inputs["src_distances"], np.float32)
    mask = np.asarray(inputs["mask"])
    dW1, db1 = np.asarray(inputs["dW1"], np.float64), np.asarray(inputs["db1"])
    dW2, db2 = np.asarray(inputs["dW2"], np.float64), np.asarray(inputs["db2"])
    dW3, db3 = np.asarray(inputs["dW3"], np.float64), np.asarray(inputs["db3"])
    dW4, db4 = np.asarray(inputs["dW4"], np.float64), np.asarray(inputs["db4"])

    assert all(np.all(b == 0) for b in (db1, db2, db3, db4)), \
        "distance-MLP collapse requires zero biases"
    assert dist.min() >= 0.0, "distance-MLP collapse requires d >= 0"
    u = np.maximum(dW1[0], 0.0)
    u = np.maximum(u @ dW2, 0.0)
    u = np.maximum(u @ dW3, 0.0)
    C = float(u @ dW4[:, 0])

    wq_b = np.asarray(inputs["Wq"], np.float32).astype(NPBF16)
    wk_b = np.asarray(inputs["Wk"], np.float32).astype(NPBF16)
    wv_b = np.asarray(inputs["Wv"], np.float32).astype(NPBF16)
    wo = np.asarray(inputs["Wo"], np.float32)
    wo8 = np.ascontiguousarray(
        wo.reshape(H // 2, 2 * DK, D).transpose(1, 0, 2)).astype(NPBF16)
    c128 = np.full((128, 1), C, np.float32)

    mf = mask != 0
    nq_max = max(int(mf[c // 4, RPC * (c % 4):RPC * (c % 4) + RPC].sum())
                 for c in range(NCORES))
    nv_max = max(int(mf[b].sum()) for b in range(B))
    NQP = max(144, 128 + ((nq_max - 128 + 15) // 16) * 16)
    NKP = max(576, 512 + ((nv_max - 512 + 63) // 64) * 64)
    KCH, POS = _chunk_layout(NQP, NKP)

    in_maps = []
    qidx_all = []
    npads = []
    for c in range(NCORES):
        b, r0 = c // 4, RPC * (c % 4)
        qidx = np.nonzero(mf[b, r0:r0 + RPC])[0]  # local valid query rows
        kid_own = r0 + qidx                       # global, matches q order
        other = np.nonzero(mf[b])[0]
        other = other[(other < r0) | (other >= r0 + RPC)]
        korder = np.concatenate([kid_own, other])
        nq, nv = len(qidx), len(korder)
        assert nq <= NQP and nv <= NKP, (nq, nv)
        qidx_all.append(qidx)
        npads.append(NKP - nv)

        qTh = np.zeros((D, NQP), NPBF16)
        qTh[:, :nq] = query[b, r0 + qidx].T.astype(NPBF16)
        kTh = np.zeros((D, NKP), NPBF16)
        kTh[:, :nv] = key[b, korder].T.astype(NPBF16)
        vTh = np.zeros((D, NKP), NPBF16)
        vTh[:, :nv] = value[b, korder].T.astype(NPBF16)
        # distances, transposed [key, query], packed into [128, 2, 512]
        dT = np.zeros((640, NQP), np.float32)
        dT[:nv, :nq] = dist[b, r0 + qidx][:, korder].T
        dpk = np.zeros((128, 2, 512), NPBF16)
        for ci, (kc0, kcn) in enumerate(KCH):
            bk, off = POS[ci]
            dpk[:kcn, bk, off:off + NQP] = dT[kc0:kc0 + kcn].astype(NPBF16)
        # zero the self-attention diagonal (key j == query j for j < NQP)
        for j in range(128):
            dpk[j, 0, j] = 0
        for p in range(NQP - 128):
            dpk[p, 0, NQP + 128 + p] = 0
        # per-key exp bias: -30 knocks zero-padded keys out of the
        # denominator (exp(0 - 30) ~ 0); chunks 0-2 are always valid
        assert nv >= 384, nv
        ebias = np.zeros((128, 2), np.float32)
        keys3 = 384 + np.arange(128)
        keys4 = 512 + np.arange(128)
        ebias[:, 0] = np.where(keys3 < nv, 0.0, -30.0)
        ebias[:, 1] = np.where(keys4 < nv, 0.0, -30.0)
        in_maps.append({
            "qT": qTh, "kT": kTh, "vT": vTh, "dist": dpk, "c128": c128,
            "wq": wq_b, "wk": wk_b, "wv": wv_b, "wo8": wo8, "ebias": ebias,
        })

    trace = os.environ.get("BASS_KERNEL_TRACE", "0") == "1"
    if trace:
        _install_ntff_hook()

    prog = _get_program(NQP, NKP)
    res = bass_utils.run_bass_kernel_spmd(
        prog, in_maps, core_ids=list(range(NCORES)), trace=trace)

    out = np.zeros((B, N, D), np.float32)
    for c in range(NCORES):
        b, r0 = c // 4, RPC * (c % 4)
        qidx = qidx_all[c]
        nq = len(qidx)
        ot = np.asarray(res.results[c]["out"], np.float32)  # [128, 2, 512]
        oT = np.concatenate(
            [ot[:, 0, 0:NQP], ot[:, 0, NQP:2 * NQP],
             ot[:, 1, 0:NQP], ot[:, 1, NQP:2 * NQP]], axis=0)  # [512, NQP]
        out[b, r0 + qidx] = oT[:, :nq].T
    kernel.last_exec_time_ns = res.exec_time_ns
    kernel._last_res = res
    kernel._last_meta = (NQP, NKP, qidx_all, npads, in_maps)
    return out


kernel.last_exec_time_ns = None
